# revision 8
# baseline (speedup 1.0000x reference)
"""Trainium2 Bass kernel for nn_CrossAttention_DenseAVInteractions (v4).

Math: the reference builds a cartesian KV grid kv[b,i,j] = pv[b,i] + pa[b,j]
over (N_v, N_a) and attends 64 queries against all N_v*N_a = 65536 keys.
Because the logits decompose as s[q,(i,j)] = (q.k_v[i]) + (q.k_a[j]), the
softmax over the product grid factorizes exactly:

    p[q,(i,j)] = softmax_i(q.k_v)[q,i] * softmax_j(q.k_a)[q,j]
    out[q]     = softmax_i(q.k_v) @ v_v + softmax_j(q.k_a) @ v_a

so the whole attention reduces to two 256-key attentions per (b, h).

Sharding (8 cores): core c handles batch b = c // 4 and the head pair
(2j, 2j+1) with j = c % 4.  Each core computes its heads' partial output
projection partial = out_heads @ Wproj[:, head_cols].T in f32; the host sums
the 4 partials per batch and adds bproj.

v4 changes (driven by the v3 NTFF profile):
 - The v3 profile showed the input stream was DMA-descriptor-rate bound:
   1536B packets at ~320ns/packet/engine => ~112 GB/s, stretching the load
   over 13us.  v4 loads the whole [128, 5376] bf16 pack in TWO DMAs split by
   partition halves (rows 0-63 on the SP queue, 64-127 on the ACT queue), so
   each descriptor covers a full 10752B row: 64 descriptors/queue, wire-rate
   bound (~24GB/s/engine x16) => all input lands ~2us after first packet.
 - All compute is scheduled once all data is resident: no per-chunk floors,
   a single dependency-ordered program with both softmax chains interleaved
   across ACT/DVE/PL so the PE never waits long.
 - Output store split by partition halves across both queues (32 descriptors
   each) instead of column halves (64 descriptors each).
 - PE warmup starts immediately (clock boost grant needs ~3.4us of PE
   activity before real matmuls).
"""

import os
import sys

import numpy as np

sys.path.insert(0, "/opt/trn_rl_repo")

import ml_dtypes

BF16 = ml_dtypes.bfloat16

DIM = 512
H = 8
HD = DIM // H          # 64
B = 2
N_MM = 64
N_A = 256
N_V = 256
SCALE = HD ** -0.5     # 0.125 (folded into Wq on the host)
N_CORES = 8

PACK_COLS = 5376

# column offsets in the packed [128, 5376] layout (all k-major tiles)
_OFF = {
    "wkv": (0, 128),      # 4 k-tiles x 128
    "xv": (512, 256),     # 4 k-tiles x 256
    "wvv": (1536, 128),
    "wva": (2048, 128),
    "wq": (2560, 128),
    "xmm": (3072, 64),
    "wka": (3328, 128),
    "xa": (3840, 256),
    "wproj": (4864, 512),  # single [128, 512] block
}

_cached = {}


def _build_program():
    import concourse.bacc as bacc
    from concourse import mybir
    from concourse.tile import TileContext

    f32 = mybir.dt.float32
    bf16 = mybir.dt.bfloat16
    nc = bacc.Bacc(name="cross_attn_dense_av")

    packA = nc.dram_tensor("packA", [128, PACK_COLS], bf16, kind="ExternalInput")
    out_d = nc.dram_tensor("out", [64, 512], f32, kind="ExternalOutput")

    from contextlib import ExitStack, contextmanager

    with TileContext(nc) as tc, ExitStack() as ctx:
        io = ctx.enter_context(tc.tile_pool(name="io", bufs=1))
        work = ctx.enter_context(tc.tile_pool(name="work", bufs=1))
        ps_k = ctx.enter_context(tc.tile_pool(name="ps_k", bufs=3, space="PSUM"))
        ps_s = ctx.enter_context(tc.tile_pool(name="ps_s", bufs=2, space="PSUM"))
        ps_qo = ctx.enter_context(tc.tile_pool(name="ps_qo", bufs=1, space="PSUM"))
        ps_f = ctx.enter_context(tc.tile_pool(name="ps_f", bufs=1, space="PSUM"))
        ps_w = ctx.enter_context(tc.tile_pool(name="ps_w", bufs=1, space="PSUM"))

        # Scheduling floors in ~x100 virtual time: floor order == per-engine
        # FIFO order the Tile scheduler emits.
        @contextmanager
        def at(us):
            with tc.tile_wait_until(us / 10.0):
                yield

        # ---- input: one [128, 5376] bf16 tile, loaded as two partition-half
        #      DMAs (full 10752B rows per descriptor, 64 descriptors each) ----
        pack_t = io.tile([128, PACK_COLS], bf16, tag="pack")
        with at(0.01):
            nc.sync.dma_start(out=pack_t[0:64, :], in_=packA[0:64, :])
        with at(0.02):
            nc.scalar.dma_start(out=pack_t[64:128, :], in_=packA[64:128, :])

        def seg(name, k=0):
            off, width = _OFF[name]
            lo = off + width * k
            return pack_t[:, lo:lo + width]

        # identity (for the normalizing transposes), built on gpsimd:
        # iota(p,f) = p - f, select == 0 from a ones tile.
        with at(0.03):
            ones = io.tile([128, 128], bf16, tag="ones")
            nc.vector.memset(ones, 1.0)
            identb = io.tile([128, 128], bf16, tag="identb")
            nc.gpsimd.affine_select(
                identb, ones, pattern=[[-1, 128]],
                compare_op=mybir.AluOpType.is_equal, fill=0.0,
                base=0, channel_multiplier=1,
            )

        # ---- PE warmup: small bf16 matmuls on memset scratch keep the PE
        #      busy through the DMA wait so the clock boost (which needs
        #      ~3.4us of activity) lands before real matmuls start ----
        with at(0.04):
            warm_sb = io.tile([128, 64], bf16, tag="warm_sb")
            nc.vector.memset(warm_sb, 0.5)
        warm_ps = ps_w.tile([64, 64], f32, tag="w_ps")
        with at(0.05):
            for w in range(44):
                nc.tensor.matmul(
                    warm_ps, warm_sb, warm_sb,
                    start=(w == 0), stop=(w == 43),
                )

        # ---- projections (all inputs resident once both DMAs land) ----
        # q first: [128(hd,2h), 64q], scale pre-folded into Wq on host
        q_ps = ps_k.tile([128, 64], f32, tag="mm")
        with at(3.00):
            for k in range(4):
                nc.tensor.matmul(
                    q_ps, seg("wq", k), seg("xmm", k),
                    start=(k == 0), stop=(k == 3),
                )
        with at(3.02):
            q2T = work.tile([128, 64], bf16, tag="q2T")
            nc.scalar.copy(q2T, q_ps)

        def kproj(wk, x, side, t, tcp):
            """kT [128ch(2 heads), 256tok] = Wk_side @ x_side.T"""
            kp = ps_k.tile([128, 256], f32, tag="mm")
            with at(t):
                for k in range(4):
                    nc.tensor.matmul(
                        kp, seg(wk, k), seg(x, k),
                        start=(k == 0), stop=(k == 3),
                    )
            with at(tcp):
                ks = work.tile([128, 256], bf16, tag=f"k_sb{side}")
                nc.vector.tensor_copy(ks, kp)
            return ks

        def scores(ks, side, ts):
            """scores (partitions = 64*h + q): per-head 64x64-tiled matmul."""
            sp = ps_s.tile([128, 256], f32, tag="spt")
            with at(ts):
                for h in range(2):
                    hs = slice(64 * h, 64 * h + 64)
                    nc.tensor.matmul(
                        sp[hs, :], q2T[hs, :], ks[hs, :],
                        start=True, stop=True, tile_position=(64 * h, 64 * h),
                    )
            return sp

        def exp_side(sp, side, texp):
            """exp (no max-subtraction: |s| < ~2.5 by construction); z via
            the activation accumulator."""
            with at(texp):
                p = work.tile([128, 256], bf16, tag=f"p{side}")
                zsum = work.tile([128, 1], f32, tag=f"zsum{side}")
                nc.scalar.activation(
                    p, sp, mybir.ActivationFunctionType.Exp, accum_out=zsum
                )
            return p, zsum

        def diag_side(zsum, side, t, eng):
            """diag(1/z) for the normalizing transpose."""
            with at(t):
                zrec = work.tile([128, 1], f32, tag=f"zrec{side}")
                diag = work.tile([128, 128], bf16, tag=f"diag{side}")
                nc.vector.reciprocal(zrec, zsum)
                getattr(nc, eng).tensor_scalar_mul(diag, identb, zrec)
            return diag

        def vproj(wv, x, side, t, tcp, cpeng):
            """v [128tok x 2 halves, 128ch] projected directly (tokens on
            partitions): v[t] = x_ktile[:, half t].T @ Wv_ktile."""
            vp = ps_k.tile([128, 2, 128], f32, tag="mm")
            with at(t):
                for th in range(2):
                    for k in range(4):
                        nc.tensor.matmul(
                            vp[:, th, :],
                            seg(x, k)[:, 128 * th:128 * th + 128],
                            seg(wv, k),
                            start=(k == 0), stop=(k == 3),
                        )
            with at(tcp):
                vs = work.tile([128, 2, 128], bf16, tag=f"v_sb{side}")
                if cpeng == "vector":
                    nc.vector.tensor_copy(vs, vp)
                else:
                    nc.scalar.copy(vs, vp)
            return vs

        def ptrans(p, diag, side, t, tcp):
            """transpose p [128(h,q), 256keys] -> [128keys, 2, (h,q)] while
            normalizing: matmul against diag(1/z) instead of the identity."""
            pt_ps = ps_s.tile([128, 2, 128], f32, tag="spt")
            with at(t):
                for th in range(2):
                    nc.tensor.matmul(
                        pt_ps[:, th, :], p[:, 128 * th:128 * th + 128], diag,
                        start=True, stop=True,
                    )
            with at(tcp):
                pt = work.tile([128, 2, 128], bf16, tag=f"pt_sb{side}")
                nc.vector.tensor_copy(pt, pt_ps)
            return pt

        # v-side chain first, a-side right behind; v projections fill the PE
        # while the exp/1/z chains run on ACT/DVE/PL.
        k_v = kproj("wkv", "xv", 0, 3.04, 3.10)
        k_a = kproj("wka", "xa", 1, 3.06, 3.14)
        sp_v = scores(k_v, 0, 3.16)
        sp_a = scores(k_a, 1, 3.18)
        p_v, zsum_v = exp_side(sp_v, 0, 3.20)
        p_a, zsum_a = exp_side(sp_a, 1, 3.24)
        v_v = vproj("wvv", "xv", 0, 3.26, 3.34, "vector")
        diag_v = diag_side(zsum_v, 0, 3.28, "vector")
        pt_v = ptrans(p_v, diag_v, 0, 3.36, 3.40)
        v_a = vproj("wva", "xa", 1, 3.38, 3.44, "scalar")
        diag_a = diag_side(zsum_a, 1, 3.42, "gpsimd")
        pt_a = ptrans(p_a, diag_a, 1, 3.46, 3.50)

        v_sb = [v_v, v_a]
        pt_sides = [pt_v, pt_a]

        # PV: o[128ch(2 heads), 64q] accumulated per head (col-tiled for h=1)
        o_ps = ps_qo.tile([128, 64], f32, tag="o")
        with at(3.54):
            for h in range(2):
                hs = slice(64 * h, 64 * h + 64)
                n = 0
                for side in range(2):
                    for t in range(2):
                        nc.tensor.matmul(
                            o_ps[hs, :],
                            v_sb[side][:, t, hs],
                            pt_sides[side][:, t, 64 * h:64 * h + 64],
                            start=(n == 0), stop=(n == 3),
                            tile_position=(0, 64 * h),
                        )
                        n += 1
        with at(3.58):
            o_sb = work.tile([128, 64], bf16, tag="o_sb")
            nc.scalar.copy(o_sb, o_ps)

        # output projection partial [64q, 512]; copies split by partition
        # halves so the two stores (one per queue) are 32 descriptors each.
        f_ps = ps_f.tile([64, 512], f32, tag="f_ps")
        f_sb = work.tile([64, 512], f32, tag="f_sb")
        with at(3.62):
            nc.tensor.matmul(
                f_ps[:, 0:256], o_sb, seg("wproj")[:, 0:256],
                start=True, stop=True,
            )
            nc.tensor.matmul(
                f_ps[:, 256:512], o_sb, seg("wproj")[:, 256:512],
                start=True, stop=True,
            )
        with at(3.66):
            nc.vector.tensor_copy(f_sb[0:32, :], f_ps[0:32, :])
            nc.scalar.copy(f_sb[32:64, :], f_ps[32:64, :])
        with at(3.70):
            nc.sync.dma_start(out=out_d[0:32, :], in_=f_sb[0:32, :])
            nc.scalar.dma_start(out=out_d[32:64, :], in_=f_sb[32:64, :])

    nc.finalize()
    return nc


def _ktiles(a):
    """[512, C] K-major -> list of 4 [128, C] k-tiles."""
    return [a[128 * k:128 * k + 128, :] for k in range(4)]


def _shard_inputs(xmm, xa, xv, Wq, Wkv, Wproj):
    """Build the 8 per-core input maps (one packed [128, 5376] bf16 tensor)."""
    in_maps = []
    for core in range(N_CORES):
        b, j = divmod(core, 4)
        r = slice(128 * j, 128 * j + 128)               # head-pair rows in [0,512)
        rv = slice(512 + 128 * j, 512 + 128 * j + 128)  # v rows in Wkv
        pack = np.concatenate(
            _ktiles(Wkv[r, :512].T)                  # wkv   @ 0
            + _ktiles(xv[b].T)                       # xv    @ 512
            + _ktiles(Wkv[rv, :512].T)               # wvv   @ 1536
            + _ktiles(Wkv[rv, 512:].T)               # wva   @ 2048
            + _ktiles((Wq[r, :] * SCALE).T)          # wq    @ 2560
            + _ktiles(xmm[b].T)                      # xmm   @ 3072
            + _ktiles(Wkv[r, 512:].T)                # wka   @ 3328
            + _ktiles(xa[b].T)                       # xa    @ 3840
            + [Wproj[:, 128 * j:128 * j + 128].T],   # wproj @ 4864
            axis=1,
        )
        assert pack.shape == (128, PACK_COLS)
        in_maps.append({"packA": np.ascontiguousarray(pack).astype(BF16)})
    return in_maps


def _get_program():
    if "nc" not in _cached:
        _cached["nc"] = _build_program()
    return _cached["nc"]


def _register_ntff_hook():
    """Best-effort: register the axon NTFF profile hook that the container's
    antenv stub doesn't provide, so run_bass_kernel_spmd(trace=True) can
    measure HW exec time. No-op on failure."""
    try:
        import types

        try:
            from antenv.axon_hooks import get_axon_ntff_profile_hook
            if get_axon_ntff_profile_hook() is not None:
                return
        except ImportError:
            pass
        import antenv
        from trn_agent_boot.trn_boot import _ntff_profile_via_ctypes

        hook = _ntff_profile_via_ctypes("/opt/axon/libaxon_pjrt.so")
        mod = types.ModuleType("antenv.axon_hooks")
        mod._hook = hook
        mod.set_axon_ntff_profile_hook = lambda h: setattr(mod, "_hook", h)
        mod.get_axon_ntff_profile_hook = lambda: mod._hook
        sys.modules["antenv.axon_hooks"] = mod
        antenv.axon_hooks = mod

        # artifact upload has no backing store in this container
        from concourse import bass_utils

        bass_utils.upload_artifacts = lambda tmpdir: tmpdir
    except Exception as e:  # pragma: no cover
        print(f"ntff hook registration failed: {e}", file=sys.stderr)


def kernel(xmm, xa, xv, Wq, Wkv, Wproj, bproj, _want_profile=False):
    from concourse.bass_utils import run_bass_kernel_spmd

    if _want_profile:
        _register_ntff_hook()
    nc = _get_program()
    in_maps = _shard_inputs(
        np.asarray(xmm, np.float32), np.asarray(xa, np.float32),
        np.asarray(xv, np.float32), np.asarray(Wq, np.float32),
        np.asarray(Wkv, np.float32), np.asarray(Wproj, np.float32),
    )
    res = run_bass_kernel_spmd(
        nc, in_maps, core_ids=list(range(N_CORES)), trace=_want_profile
    )
    out = np.zeros((B, N_MM, DIM), np.float32)
    for core in range(N_CORES):
        out[core // 4] += res.results[core]["out"]
    out += np.asarray(bproj, np.float32)[None, None, :]
    if _want_profile:
        return out, res
    return out


# revision 9
# speedup vs baseline: 1.1264x; 1.1264x over previous
"""Trainium2 Bass kernel for nn_CrossAttention_DenseAVInteractions (v4).

Math: the reference builds a cartesian KV grid kv[b,i,j] = pv[b,i] + pa[b,j]
over (N_v, N_a) and attends 64 queries against all N_v*N_a = 65536 keys.
Because the logits decompose as s[q,(i,j)] = (q.k_v[i]) + (q.k_a[j]), the
softmax over the product grid factorizes exactly:

    p[q,(i,j)] = softmax_i(q.k_v)[q,i] * softmax_j(q.k_a)[q,j]
    out[q]     = softmax_i(q.k_v) @ v_v + softmax_j(q.k_a) @ v_a

so the whole attention reduces to two 256-key attentions per (b, h).

Sharding (8 cores): core c handles batch b = c // 4 and the head pair
(2j, 2j+1) with j = c % 4.  Each core computes its heads' partial output
projection partial = out_heads @ Wproj[:, head_cols].T in f32; the host sums
the 4 partials per batch and adds bproj.

v4 changes (driven by the v3 NTFF profile):
 - The v3 profile showed the input stream was DMA-descriptor-rate bound:
   1536B packets at ~320ns/packet/engine => ~112 GB/s, stretching the load
   over 13us.  v4 loads the whole [128, 5376] bf16 pack in TWO DMAs split by
   partition halves (rows 0-63 on the SP queue, 64-127 on the ACT queue), so
   each descriptor covers a full 10752B row: 64 descriptors/queue, wire-rate
   bound (~24GB/s/engine x16) => all input lands ~2us after first packet.
 - All compute is scheduled once all data is resident: no per-chunk floors,
   a single dependency-ordered program with both softmax chains interleaved
   across ACT/DVE/PL so the PE never waits long.
 - Output store split by partition halves across both queues (32 descriptors
   each) instead of column halves (64 descriptors each).
 - PE warmup starts immediately (clock boost grant needs ~3.4us of PE
   activity before real matmuls).
"""

import os
import sys

import numpy as np

sys.path.insert(0, "/opt/trn_rl_repo")

import ml_dtypes

BF16 = ml_dtypes.bfloat16

DIM = 512
H = 8
HD = DIM // H          # 64
B = 2
N_MM = 64
N_A = 256
N_V = 256
SCALE = HD ** -0.5     # 0.125 (folded into Wq on the host)
N_CORES = 8

PACK_COLS = 5504

# column offsets in the packed [128, 5376] layout (all k-major tiles)
_OFF = {
    "wkv": (0, 128),      # 4 k-tiles x 128
    "xv": (512, 256),     # 4 k-tiles x 256
    "wvv": (1536, 128),
    "wva": (2048, 128),
    "wq": (2560, 128),
    "xmm": (3072, 64),
    "wka": (3328, 128),
    "xa": (3840, 256),
    "wproj": (4864, 512),  # single [128, 512] block
    "ident": (5376, 128),  # bf16 identity for the transposes
}

_cached = {}


def _build_program():
    import concourse.bacc as bacc
    from concourse import mybir
    from concourse.tile import TileContext

    f32 = mybir.dt.float32
    bf16 = mybir.dt.bfloat16
    nc = bacc.Bacc(name="cross_attn_dense_av")

    packA = nc.dram_tensor("packA", [128, PACK_COLS], bf16, kind="ExternalInput")
    out_d = nc.dram_tensor("out", [64, 512], f32, kind="ExternalOutput")

    from contextlib import ExitStack, contextmanager

    with TileContext(nc) as tc, ExitStack() as ctx:
        io = ctx.enter_context(tc.tile_pool(name="io", bufs=1))
        work = ctx.enter_context(tc.tile_pool(name="work", bufs=1))
        ps_k = ctx.enter_context(tc.tile_pool(name="ps_k", bufs=3, space="PSUM"))
        ps_s = ctx.enter_context(tc.tile_pool(name="ps_s", bufs=2, space="PSUM"))
        ps_qo = ctx.enter_context(tc.tile_pool(name="ps_qo", bufs=1, space="PSUM"))
        ps_f = ctx.enter_context(tc.tile_pool(name="ps_f", bufs=1, space="PSUM"))
        ps_w = ctx.enter_context(tc.tile_pool(name="ps_w", bufs=1, space="PSUM"))

        # Scheduling floors in ~x100 virtual time: floor order == per-engine
        # FIFO order the Tile scheduler emits.
        @contextmanager
        def at(us):
            with tc.tile_wait_until(us / 10.0):
                yield

        # ---- input: one [128, 5376] bf16 tile, loaded as two partition-half
        #      DMAs (full 10752B rows per descriptor, 64 descriptors each) ----
        pack_t = io.tile([128, PACK_COLS], bf16, tag="pack")
        with at(0.01):
            nc.sync.dma_start(out=pack_t[0:64, :], in_=packA[0:64, :])
        with at(0.02):
            nc.scalar.dma_start(out=pack_t[64:128, :], in_=packA[64:128, :])

        def seg(name, k=0):
            off, width = _OFF[name]
            lo = off + width * k
            return pack_t[:, lo:lo + width]

        # identity (for the normalizing transposes) ships in the pack
        identb = seg("ident")

        # ---- PE warmup: small bf16 matmuls on memset scratch keep the PE
        #      busy through the DMA wait so the clock boost (which needs
        #      ~3.4us of activity) lands before real matmuls start ----
        with at(0.04):
            warm_sb = io.tile([128, 256], bf16, tag="warm_sb")
            nc.vector.memset(warm_sb, 0.5)
        warm_ps = ps_w.tile([128, 256], f32, tag="w_ps")
        with at(0.05):
            for w in range(36):
                nc.tensor.matmul(
                    warm_ps, warm_sb[:, 0:128], warm_sb,
                    start=(w == 0), stop=(w == 35),
                )

        # ---- projections (all inputs resident once both DMAs land) ----
        # q first: [128(hd,2h), 64q], scale pre-folded into Wq on host
        q_ps = ps_k.tile([128, 64], f32, tag="mm")
        with at(3.00):
            for k in range(4):
                nc.tensor.matmul(
                    q_ps, seg("wq", k), seg("xmm", k),
                    start=(k == 0), stop=(k == 3),
                )
        with at(3.02):
            q2T = work.tile([128, 64], bf16, tag="q2T")
            nc.scalar.copy(q2T, q_ps)

        def kproj(wk, x, side, t, tcp):
            """kT [128ch(2 heads), 256tok] = Wk_side @ x_side.T"""
            kp = ps_k.tile([128, 256], f32, tag="mm")
            with at(t):
                for k in range(4):
                    nc.tensor.matmul(
                        kp, seg(wk, k), seg(x, k),
                        start=(k == 0), stop=(k == 3),
                    )
            with at(tcp):
                ks = work.tile([128, 256], bf16, tag=f"k_sb{side}")
                nc.vector.tensor_copy(ks, kp)
            return ks

        def scores(ks, side, ts):
            """scores (partitions = 64*h + q): per-head 64x64-tiled matmul."""
            sp = ps_s.tile([128, 256], f32, tag="spt")
            with at(ts):
                for h in range(2):
                    hs = slice(64 * h, 64 * h + 64)
                    nc.tensor.matmul(
                        sp[hs, :], q2T[hs, :], ks[hs, :],
                        start=True, stop=True, tile_position=(64 * h, 64 * h),
                    )
            return sp

        def exp_side(sp, side, texp):
            """exp (no max-subtraction: |s| < ~2.5 by construction); z via
            the activation accumulator."""
            with at(texp):
                p = work.tile([128, 256], bf16, tag=f"p{side}")
                zsum = work.tile([128, 1], f32, tag=f"zsum{side}")
                nc.scalar.activation(
                    p, sp, mybir.ActivationFunctionType.Exp, accum_out=zsum
                )
            return p, zsum

        def diag_side(zsum, side, t, eng):
            """diag(1/z) for the normalizing transpose."""
            with at(t):
                zrec = work.tile([128, 1], f32, tag=f"zrec{side}")
                diag = work.tile([128, 128], bf16, tag=f"diag{side}")
                nc.vector.reciprocal(zrec, zsum)
                getattr(nc, eng).tensor_scalar_mul(diag, identb, zrec)
            return diag

        def vproj(wv, x, side, t, tcp, cpeng):
            """v [128tok x 2 halves, 128ch] projected directly (tokens on
            partitions): v[t] = x_ktile[:, half t].T @ Wv_ktile."""
            vp = ps_k.tile([128, 2, 128], f32, tag="mm")
            with at(t):
                for th in range(2):
                    for k in range(4):
                        nc.tensor.matmul(
                            vp[:, th, :],
                            seg(x, k)[:, 128 * th:128 * th + 128],
                            seg(wv, k),
                            start=(k == 0), stop=(k == 3),
                        )
            with at(tcp):
                vs = work.tile([128, 2, 128], bf16, tag=f"v_sb{side}")
                if cpeng == "vector":
                    nc.vector.tensor_copy(vs, vp)
                else:
                    nc.scalar.copy(vs, vp)
            return vs

        def ptrans(p, diag, side, t, tcp):
            """transpose p [128(h,q), 256keys] -> [128keys, 2, (h,q)] while
            normalizing: matmul against diag(1/z) instead of the identity."""
            pt_ps = ps_s.tile([128, 2, 128], f32, tag="spt")
            with at(t):
                for th in range(2):
                    nc.tensor.matmul(
                        pt_ps[:, th, :], p[:, 128 * th:128 * th + 128], diag,
                        start=True, stop=True,
                    )
            with at(tcp):
                pt = work.tile([128, 2, 128], bf16, tag=f"pt_sb{side}")
                nc.vector.tensor_copy(pt, pt_ps)
            return pt

        # v-side chain first, a-side right behind; v projections fill the PE
        # while the exp/1/z chains run on ACT/DVE/PL.
        k_v = kproj("wkv", "xv", 0, 3.04, 3.10)
        k_a = kproj("wka", "xa", 1, 3.06, 3.14)
        sp_v = scores(k_v, 0, 3.16)
        sp_a = scores(k_a, 1, 3.18)
        p_v, zsum_v = exp_side(sp_v, 0, 3.20)
        p_a, zsum_a = exp_side(sp_a, 1, 3.24)
        v_v = vproj("wvv", "xv", 0, 3.26, 3.34, "vector")
        diag_v = diag_side(zsum_v, 0, 3.28, "vector")
        pt_v = ptrans(p_v, diag_v, 0, 3.36, 3.40)
        v_a = vproj("wva", "xa", 1, 3.38, 3.44, "scalar")
        diag_a = diag_side(zsum_a, 1, 3.42, "vector")
        pt_a = ptrans(p_a, diag_a, 1, 3.46, 3.50)

        v_sb = [v_v, v_a]
        pt_sides = [pt_v, pt_a]

        # PV: o[128ch(2 heads), 64q] accumulated per head (col-tiled for h=1)
        o_ps = ps_qo.tile([128, 64], f32, tag="o")
        with at(3.54):
            for h in range(2):
                hs = slice(64 * h, 64 * h + 64)
                n = 0
                for side in range(2):
                    for t in range(2):
                        nc.tensor.matmul(
                            o_ps[hs, :],
                            v_sb[side][:, t, hs],
                            pt_sides[side][:, t, 64 * h:64 * h + 64],
                            start=(n == 0), stop=(n == 3),
                            tile_position=(0, 64 * h),
                        )
                        n += 1
        with at(3.58):
            o_sb = work.tile([128, 64], bf16, tag="o_sb")
            nc.scalar.copy(o_sb, o_ps)

        # output projection partial [64q, 512]; copies split by partition
        # halves so the two stores (one per queue) are 32 descriptors each.
        f_ps = ps_f.tile([64, 512], f32, tag="f_ps")
        f_sb = work.tile([64, 512], f32, tag="f_sb")
        with at(3.62):
            nc.tensor.matmul(
                f_ps[:, 0:256], o_sb, seg("wproj")[:, 0:256],
                start=True, stop=True,
            )
            nc.tensor.matmul(
                f_ps[:, 256:512], o_sb, seg("wproj")[:, 256:512],
                start=True, stop=True,
            )
        with at(3.66):
            nc.vector.tensor_copy(f_sb[:, 0:256], f_ps[:, 0:256])
            nc.scalar.copy(f_sb[:, 256:512], f_ps[:, 256:512])
        with at(3.70):
            nc.sync.dma_start(out=out_d[0:32, :], in_=f_sb[0:32, :])
            nc.scalar.dma_start(out=out_d[32:64, :], in_=f_sb[32:64, :])

    nc.finalize()
    return nc


def _ktiles(a):
    """[512, C] K-major -> list of 4 [128, C] k-tiles."""
    return [a[128 * k:128 * k + 128, :] for k in range(4)]


def _shard_inputs(xmm, xa, xv, Wq, Wkv, Wproj):
    """Build the 8 per-core input maps (one packed [128, 5376] bf16 tensor)."""
    in_maps = []
    for core in range(N_CORES):
        b, j = divmod(core, 4)
        r = slice(128 * j, 128 * j + 128)               # head-pair rows in [0,512)
        rv = slice(512 + 128 * j, 512 + 128 * j + 128)  # v rows in Wkv
        pack = np.concatenate(
            _ktiles(Wkv[r, :512].T)                  # wkv   @ 0
            + _ktiles(xv[b].T)                       # xv    @ 512
            + _ktiles(Wkv[rv, :512].T)               # wvv   @ 1536
            + _ktiles(Wkv[rv, 512:].T)               # wva   @ 2048
            + _ktiles((Wq[r, :] * SCALE).T)          # wq    @ 2560
            + _ktiles(xmm[b].T)                      # xmm   @ 3072
            + _ktiles(Wkv[r, 512:].T)                # wka   @ 3328
            + _ktiles(xa[b].T)                       # xa    @ 3840
            + [Wproj[:, 128 * j:128 * j + 128].T,    # wproj @ 4864
               np.eye(128, dtype=np.float32)],       # ident @ 5376
            axis=1,
        )
        assert pack.shape == (128, PACK_COLS)
        in_maps.append({"packA": np.ascontiguousarray(pack).astype(BF16)})
    return in_maps


def _get_program():
    if "nc" not in _cached:
        _cached["nc"] = _build_program()
    return _cached["nc"]


def _register_ntff_hook():
    """Best-effort: register the axon NTFF profile hook that the container's
    antenv stub doesn't provide, so run_bass_kernel_spmd(trace=True) can
    measure HW exec time. No-op on failure."""
    try:
        import types

        try:
            from antenv.axon_hooks import get_axon_ntff_profile_hook
            if get_axon_ntff_profile_hook() is not None:
                return
        except ImportError:
            pass
        import antenv
        from trn_agent_boot.trn_boot import _ntff_profile_via_ctypes

        hook = _ntff_profile_via_ctypes("/opt/axon/libaxon_pjrt.so")
        mod = types.ModuleType("antenv.axon_hooks")
        mod._hook = hook
        mod.set_axon_ntff_profile_hook = lambda h: setattr(mod, "_hook", h)
        mod.get_axon_ntff_profile_hook = lambda: mod._hook
        sys.modules["antenv.axon_hooks"] = mod
        antenv.axon_hooks = mod

        # artifact upload has no backing store in this container
        from concourse import bass_utils

        bass_utils.upload_artifacts = lambda tmpdir: tmpdir
    except Exception as e:  # pragma: no cover
        print(f"ntff hook registration failed: {e}", file=sys.stderr)


def kernel(xmm, xa, xv, Wq, Wkv, Wproj, bproj, _want_profile=False):
    from concourse.bass_utils import run_bass_kernel_spmd

    if _want_profile:
        _register_ntff_hook()
    nc = _get_program()
    in_maps = _shard_inputs(
        np.asarray(xmm, np.float32), np.asarray(xa, np.float32),
        np.asarray(xv, np.float32), np.asarray(Wq, np.float32),
        np.asarray(Wkv, np.float32), np.asarray(Wproj, np.float32),
    )
    res = run_bass_kernel_spmd(
        nc, in_maps, core_ids=list(range(N_CORES)), trace=_want_profile
    )
    out = np.zeros((B, N_MM, DIM), np.float32)
    for core in range(N_CORES):
        out[core // 4] += res.results[core]["out"]
    out += np.asarray(bproj, np.float32)[None, None, :]
    if _want_profile:
        return out, res
    return out


# revision 11
# speedup vs baseline: 1.3041x; 1.1578x over previous
"""Trainium2 Bass kernel for nn_CrossAttention_DenseAVInteractions (v6).

Math: the reference builds a cartesian KV grid kv[b,i,j] = pv[b,i] + pa[b,j]
over (N_v, N_a) and attends 64 queries against all N_v*N_a = 65536 keys.
Because the logits decompose as s[q,(i,j)] = (q.k_v[i]) + (q.k_a[j]), the
softmax over the product grid factorizes exactly:

    p[q,(i,j)] = softmax_i(q.k_v)[q,i] * softmax_j(q.k_a)[q,j]
    out[q]     = softmax_i(q.k_v) @ v_v + softmax_j(q.k_a) @ v_a

so the whole attention reduces to two 256-key attentions per (b, h).

Sharding (8 cores): core c handles batch b = c // 4 and the head pair
(2j, 2j+1) with j = c % 4.  Each core computes its heads' partial output
projection partial = out_heads @ Wproj[:, head_cols].T in f32; the host sums
the 4 partials per batch and adds bproj.

v6 design notes (from the v4/v5 NTFF profiles):
 - The two HWDGE queues (SP/ACT) share ONE descriptor generator and drain
   doorbells in order, so a second queue adds no bandwidth - only postamble
   cost.  ALL DMA goes on the SP queue; the ACT HWDGE queue and the unused
   SWDGE (gpsimd) queue declarations are pruned from the module (the NEFF
   epilogue's per-engine semaphore chain scales with declared queues).
 - Descriptor economics: ~20ns/descriptor generation, ~14.3 GB/s/engine wire
   over 16 engines (~229 GB/s aggregate).  One descriptor per SBUF partition
   row, so wide rows amortize generation.  The input pack is split in two
   wide chunks in consumption order: A = both K-side operand sets + q
   (7680B rows), B = V weights + Wproj + identity (3328B rows), into
   SEPARATE tiles (two DMAs writing one tile get WAW-serialized by the Tile
   framework).
 - Tail: the critical chain is the second exp (a-side) -> 1/z -> diag ->
   normalizing transpose -> PV -> out-projection.  DVE runs the diag chain
   ahead of the bulky pt casts; ACT takes the v copies after the exps; the
   PV accumulation is ordered side-major so the v-side half issues early.
 - f copies go to two separate SBUF tiles (partition-split) so the two
   copy->store chains are independent; stores are 32 descriptors each.
"""

import os
import sys

import numpy as np

sys.path.insert(0, "/opt/trn_rl_repo")

import ml_dtypes

BF16 = ml_dtypes.bfloat16

DIM = 512
H = 8
HD = DIM // H          # 64
B = 2
N_MM = 64
N_A = 256
N_V = 256
SCALE = HD ** -0.5     # 0.125 (folded into Wq on the host)
N_CORES = 8

# chunk A: both K-side operand sets + q inputs (consumption order)
# chunk B: V weights + Wproj + identity
COLS_A = 3840
COLS_B = 1664
PACK_COLS = COLS_A + COLS_B  # 5504

# (chunk, column offset in chunk, tile width)
_OFF = {
    "wkv": ("A", 0, 128),      # 4 k-tiles x 128
    "xv": ("A", 512, 256),     # 4 k-tiles x 256
    "wka": ("A", 1536, 128),
    "xa": ("A", 2048, 256),
    "wq": ("A", 3072, 128),
    "xmm": ("A", 3584, 64),
    "wvv": ("B", 0, 128),
    "wva": ("B", 512, 128),
    "wproj": ("B", 1024, 512),
    "ident": ("B", 1536, 128),
}

_cached = {}


def _build_program():
    import concourse.bacc as bacc
    from concourse import mybir
    from concourse.tile import TileContext

    f32 = mybir.dt.float32
    bf16 = mybir.dt.bfloat16
    nc = bacc.Bacc(name="cross_attn_dense_av")

    packA = nc.dram_tensor("packA", [128, PACK_COLS], bf16, kind="ExternalInput")
    out_d = nc.dram_tensor("out", [64, 512], f32, kind="ExternalOutput")

    from contextlib import ExitStack, contextmanager

    with TileContext(nc) as tc, ExitStack() as ctx:
        io = ctx.enter_context(tc.tile_pool(name="io", bufs=1))
        work = ctx.enter_context(tc.tile_pool(name="work", bufs=1))
        ps_k = ctx.enter_context(tc.tile_pool(name="ps_k", bufs=3, space="PSUM"))
        ps_s = ctx.enter_context(tc.tile_pool(name="ps_s", bufs=2, space="PSUM"))
        ps_qo = ctx.enter_context(tc.tile_pool(name="ps_qo", bufs=1, space="PSUM"))
        ps_f = ctx.enter_context(tc.tile_pool(name="ps_f", bufs=1, space="PSUM"))
        ps_w = ctx.enter_context(tc.tile_pool(name="ps_w", bufs=1, space="PSUM"))

        # Scheduling floors in ~x100 virtual time: floor order == per-engine
        # FIFO order the Tile scheduler emits.
        @contextmanager
        def at(us):
            with tc.tile_wait_until(us / 10.0):
                yield

        # ---- input: two wide chunks, both on the SP HWDGE queue, separate
        #      tiles (A gates the K chains; B holds the late V-side data) ----
        with at(0.01):
            pack_a = io.tile([128, COLS_A], bf16, tag="packa")
            nc.sync.dma_start(out=pack_a, in_=packA[:, 0:COLS_A])
        with at(0.02):
            pack_b = io.tile([128, COLS_B], bf16, tag="packb")
            nc.sync.dma_start(out=pack_b, in_=packA[:, COLS_A:PACK_COLS])
        chunk_t = {"A": pack_a, "B": pack_b}

        def seg(name, k=0):
            chunk, off, width = _OFF[name]
            lo = off + width * k
            return chunk_t[chunk][:, lo:lo + width]

        identb = seg("ident")

        # ---- PE warmup: matmuls on memset scratch keep the PE busy through
        #      the DMA wait so the clock boost (which needs a few us of
        #      activity) lands before real matmuls start ----
        with at(0.03):
            warm_sb = io.tile([128, 256], bf16, tag="warm_sb")
            nc.vector.memset(warm_sb, 0.5)
        warm_ps = ps_w.tile([128, 256], f32, tag="w_ps")
        with at(0.05):
            for w in range(30):
                nc.tensor.matmul(
                    warm_ps, warm_sb[:, 0:128], warm_sb,
                    start=(w == 0), stop=(w == 29),
                )

        # ---- compute (chunk A resident) ----
        # q: [128(hd,2h), 64q], scale pre-folded into Wq on host
        q_ps = ps_k.tile([128, 64], f32, tag="mm")
        with at(3.00):
            for k in range(4):
                nc.tensor.matmul(
                    q_ps, seg("wq", k), seg("xmm", k),
                    start=(k == 0), stop=(k == 3),
                )
        with at(3.02):
            q2T = work.tile([128, 64], bf16, tag="q2T")
            nc.scalar.copy(q2T, q_ps)

        def kproj(wk, x, side, t, tcp):
            """kT [128ch(2 heads), 256tok] = Wk_side @ x_side.T"""
            kp = ps_k.tile([128, 256], f32, tag="mm")
            with at(t):
                for k in range(4):
                    nc.tensor.matmul(
                        kp, seg(wk, k), seg(x, k),
                        start=(k == 0), stop=(k == 3),
                    )
            with at(tcp):
                ks = work.tile([128, 256], bf16, tag=f"k_sb{side}")
                nc.vector.tensor_copy(ks, kp)
            return ks

        def scores(ks, side, ts):
            """scores (partitions = 64*h + q): per-head 64x64-tiled matmul."""
            sp = ps_s.tile([128, 256], f32, tag="spt")
            with at(ts):
                for h in range(2):
                    hs = slice(64 * h, 64 * h + 64)
                    nc.tensor.matmul(
                        sp[hs, :], q2T[hs, :], ks[hs, :],
                        start=True, stop=True, tile_position=(64 * h, 64 * h),
                    )
            return sp

        def exp_side(sp, side, texp):
            """exp (no max-subtraction: |s| < ~2.5 by construction); z via
            the activation accumulator."""
            with at(texp):
                p = work.tile([128, 256], bf16, tag=f"p{side}")
                zsum = work.tile([128, 1], f32, tag=f"zsum{side}")
                nc.scalar.activation(
                    p, sp, mybir.ActivationFunctionType.Exp, accum_out=zsum
                )
            return p, zsum

        def diag_side(zsum, side, t):
            """diag(1/z) for the normalizing transpose (DVE)."""
            with at(t):
                zrec = work.tile([128, 1], f32, tag=f"zrec{side}")
                diag = work.tile([128, 128], bf16, tag=f"diag{side}")
                nc.vector.reciprocal(zrec, zsum)
                nc.vector.tensor_scalar_mul(diag, identb, zrec)
            return diag

        def vproj(wv, x, side, t, tcp):
            """v [128tok x 2 halves, 128ch] projected directly (tokens on
            partitions): v[t] = x_ktile[:, half t].T @ Wv_ktile.  Copies on
            ACT (after the exps)."""
            vp = ps_k.tile([128, 2, 128], f32, tag="mm")
            with at(t):
                for th in range(2):
                    for k in range(4):
                        nc.tensor.matmul(
                            vp[:, th, :],
                            seg(x, k)[:, 128 * th:128 * th + 128],
                            seg(wv, k),
                            start=(k == 0), stop=(k == 3),
                        )
            with at(tcp):
                vs = work.tile([128, 2, 128], bf16, tag=f"v_sb{side}")
                nc.scalar.copy(vs, vp)
            return vs

        def ptrans(p, diag, side, t, tcp):
            """transpose p [128(h,q), 256keys] -> [128keys, 2, (h,q)] while
            normalizing: matmul against diag(1/z) instead of the identity."""
            pt_ps = ps_s.tile([128, 2, 128], f32, tag="spt")
            with at(t):
                for th in range(2):
                    nc.tensor.matmul(
                        pt_ps[:, th, :], p[:, 128 * th:128 * th + 128], diag,
                        start=True, stop=True,
                    )
            with at(tcp):
                pt = work.tile([128, 2, 128], bf16, tag=f"pt_sb{side}")
                nc.vector.tensor_copy(pt, pt_ps)
            return pt

        k_v = kproj("wkv", "xv", 0, 3.04, 3.06)
        k_a = kproj("wka", "xa", 1, 3.08, 3.10)
        sp_v = scores(k_v, 0, 3.12)
        p_v, zsum_v = exp_side(sp_v, 0, 3.14)
        sp_a = scores(k_a, 1, 3.16)
        p_a, zsum_a = exp_side(sp_a, 1, 3.18)
        v_v = vproj("wvv", "xv", 0, 3.20, 3.28)
        v_a = vproj("wva", "xa", 1, 3.22, 3.30)
        diag_v = diag_side(zsum_v, 0, 3.24)
        diag_a = diag_side(zsum_a, 1, 3.26)
        pt_v = ptrans(p_v, diag_v, 0, 3.32, 3.34)
        pt_a = ptrans(p_a, diag_a, 1, 3.36, 3.38)

        v_sb = [v_v, v_a]
        pt_sides = [pt_v, pt_a]

        # PV: o[128ch(2 heads), 64q], side-major so the v-side half issues as
        # soon as pt_v/v_v land; per-head accumulation spans both sides.
        o_ps = ps_qo.tile([128, 64], f32, tag="o")
        for side, t in ((0, 3.40), (1, 3.42)):
            with at(t):
                for h in range(2):
                    hs = slice(64 * h, 64 * h + 64)
                    for tt in range(2):
                        nc.tensor.matmul(
                            o_ps[hs, :],
                            v_sb[side][:, tt, hs],
                            pt_sides[side][:, tt, 64 * h:64 * h + 64],
                            start=(side == 0 and tt == 0),
                            stop=(side == 1 and tt == 1),
                            tile_position=(0, 64 * h),
                        )
        with at(3.44):
            o_sb = work.tile([128, 64], bf16, tag="o_sb")
            nc.scalar.copy(o_sb, o_ps)

        # output projection partial [64q, 512]: two column-half matmuls, then
        # two independent partition-half copy->store chains (separate tiles).
        f_ps = ps_f.tile([64, 512], f32, tag="f_ps")
        with at(3.46):
            nc.tensor.matmul(
                f_ps[:, 0:256], o_sb, seg("wproj")[:, 0:256],
                start=True, stop=True,
            )
            nc.tensor.matmul(
                f_ps[:, 256:512], o_sb, seg("wproj")[:, 256:512],
                start=True, stop=True,
            )
        f_lo = work.tile([64, 512], f32, tag="f_lo")
        f_hi = work.tile([64, 512], f32, tag="f_hi")
        with at(3.48):
            nc.vector.tensor_copy(f_lo[0:32, :], f_ps[0:32, :])
            nc.scalar.copy(f_hi[32:64, :], f_ps[32:64, :])
        with at(3.52):
            nc.sync.dma_start(out=out_d[0:32, :], in_=f_lo[0:32, :])
        with at(3.54):
            nc.sync.dma_start(out=out_d[32:64, :], in_=f_hi[32:64, :])

    # All DMA runs on the SP HWDGE queue; prune the unused ACT HWDGE and
    # SWDGE (Pool) queue declarations - the NEFF epilogue's per-engine
    # semaphore chain scales with the number of declared DMA queues.
    nc.m.queues = [q for q in nc.m.queues if q.name == "qSPDynamicHW"]

    nc.finalize()
    return nc


def _ktiles(a):
    """[512, C] K-major -> list of 4 [128, C] k-tiles."""
    return [a[128 * k:128 * k + 128, :] for k in range(4)]


def _shard_inputs(xmm, xa, xv, Wq, Wkv, Wproj):
    """Build the 8 per-core input maps (one packed [128, 5504] bf16 tensor)."""
    in_maps = []
    for core in range(N_CORES):
        b, j = divmod(core, 4)
        r = slice(128 * j, 128 * j + 128)               # head-pair rows in [0,512)
        rv = slice(512 + 128 * j, 512 + 128 * j + 128)  # v rows in Wkv
        pack = np.concatenate(
            _ktiles(Wkv[r, :512].T)                  # wkv   A@0
            + _ktiles(xv[b].T)                       # xv    A@512
            + _ktiles(Wkv[r, 512:].T)                # wka   A@1536
            + _ktiles(xa[b].T)                       # xa    A@2048
            + _ktiles((Wq[r, :] * SCALE).T)          # wq    A@3072
            + _ktiles(xmm[b].T)                      # xmm   A@3584
            + _ktiles(Wkv[rv, :512].T)               # wvv   B@0
            + _ktiles(Wkv[rv, 512:].T)               # wva   B@512
            + [Wproj[:, 128 * j:128 * j + 128].T,    # wproj B@1024
               np.eye(128, dtype=np.float32)],       # ident B@1536
            axis=1,
        )
        assert pack.shape == (128, PACK_COLS)
        in_maps.append({"packA": np.ascontiguousarray(pack).astype(BF16)})
    return in_maps


def _get_program():
    if "nc" not in _cached:
        _cached["nc"] = _build_program()
    return _cached["nc"]


def _register_ntff_hook():
    """Best-effort: register the axon NTFF profile hook that the container's
    antenv stub doesn't provide, so run_bass_kernel_spmd(trace=True) can
    measure HW exec time. No-op on failure."""
    try:
        import types

        try:
            from antenv.axon_hooks import get_axon_ntff_profile_hook
            if get_axon_ntff_profile_hook() is not None:
                return
        except ImportError:
            pass
        import antenv
        from trn_agent_boot.trn_boot import _ntff_profile_via_ctypes

        hook = _ntff_profile_via_ctypes("/opt/axon/libaxon_pjrt.so")
        mod = types.ModuleType("antenv.axon_hooks")
        mod._hook = hook
        mod.set_axon_ntff_profile_hook = lambda h: setattr(mod, "_hook", h)
        mod.get_axon_ntff_profile_hook = lambda: mod._hook
        sys.modules["antenv.axon_hooks"] = mod
        antenv.axon_hooks = mod

        # artifact upload has no backing store in this container
        from concourse import bass_utils

        bass_utils.upload_artifacts = lambda tmpdir: tmpdir
    except Exception as e:  # pragma: no cover
        print(f"ntff hook registration failed: {e}", file=sys.stderr)


def kernel(xmm, xa, xv, Wq, Wkv, Wproj, bproj, _want_profile=False):
    from concourse.bass_utils import run_bass_kernel_spmd

    if _want_profile:
        _register_ntff_hook()
    nc = _get_program()
    in_maps = _shard_inputs(
        np.asarray(xmm, np.float32), np.asarray(xa, np.float32),
        np.asarray(xv, np.float32), np.asarray(Wq, np.float32),
        np.asarray(Wkv, np.float32), np.asarray(Wproj, np.float32),
    )
    res = run_bass_kernel_spmd(
        nc, in_maps, core_ids=list(range(N_CORES)), trace=_want_profile
    )
    out = np.zeros((B, N_MM, DIM), np.float32)
    for core in range(N_CORES):
        out[core // 4] += res.results[core]["out"]
    out += np.asarray(bproj, np.float32)[None, None, :]
    if _want_profile:
        return out, res
    return out
